# revision 15
# baseline (speedup 1.0000x reference)
"""Trainium2 Bass kernel for nn_EnhancedDragGNN (GNN message passing).

Self-contained: takes FULL inputs (x [100000,3] f32, edge_index [2,800000] int,
batch [100000] int, params dict), distributes across 8 NeuronCores, returns
FULL output [8,1] f32.

Sharding: nodes sharded contiguously across 8 cores; edges owned by their dst
core and sorted by dst tile; per-layer feature tables AllGathered; per-edge
gathers via the Q7 dma_gather custom DMA; segment-sums via one-hot matmuls on
the tensor engine (PSUM-accumulated per 128-node tile). GAT softmax uses a
per-dst upper-bound shift (exactly equivalent to segment_max by shift
invariance of softmax ratios).
"""
import math
import numpy as np
from contextlib import ExitStack
from dataclasses import dataclass


@dataclass
class Cfg:
    N: int = 100000
    E: int = 800000
    NC: int = 8
    G: int = 8
    TILE: int = 128
    WIN: int = 8

    @property
    def SHARD(self):
        return self.N // self.NC

    @property
    def NTILES(self):
        return (self.SHARD + self.TILE - 1) // self.TILE

    @property
    def SHARD_P(self):
        return self.NTILES * self.TILE

    @property
    def NSEG(self):
        return max(1, math.ceil(self.NC * self.SHARD_P / 32000))

    @property
    def SEGSZ(self):
        npad = self.NC * self.SHARD_P
        return math.ceil(npad / self.NSEG / self.SHARD_P) * self.SHARD_P

    @property
    def NPAD(self):
        return self.NC * self.SHARD_P


def preprocess(cfg, edge_index):
    N, NC, TILE = cfg.N, cfg.NC, cfg.TILE
    SHARD, SHARD_P, SEGSZ = cfg.SHARD, cfg.SHARD_P, cfg.SEGSZ
    NT, NSEG = cfg.NTILES, cfg.NSEG

    src = np.concatenate([edge_index[0], np.arange(N)]).astype(np.int64)
    dst = np.concatenate([edge_index[1], np.arange(N)]).astype(np.int64)
    src_g = (src // SHARD) * SHARD_P + (src % SHARD)
    core_of = dst // SHARD
    loc = dst % SHARD
    tile_of = loc // TILE
    dloc = loc % TILE
    seg_of = src_g // SEGSZ

    counts = np.zeros((NC, NT, NSEG), dtype=np.int64)
    np.add.at(counts, (core_of, tile_of, seg_of), 1)
    padded = ((counts.max(axis=0) + TILE - 1) // TILE) * TILE
    run_off = np.zeros(NT * NSEG, dtype=np.int64)
    flat = padded.reshape(-1)
    run_off[1:] = np.cumsum(flat)[:-1]
    stream = int(flat.sum())
    nchunks = stream // TILE
    tile_of_chunk = np.repeat(np.arange(NT), padded.sum(axis=1) // TILE)

    WINE = cfg.WIN * TILE
    runs = []
    for t in range(NT):
        for s in range(NSEG):
            off, n = int(run_off[t * NSEG + s]), int(padded[t, s])
            while n > 0:
                room = WINE - (off % WINE)
                take = min(n, room, 1024)
                runs.append((off, take, s))
                off += take
                n -= take

    order = np.lexsort((seg_of, tile_of, core_of))
    percore = []
    for c in range(NC):
        o = order[core_of[order] == c]
        srcl = np.zeros(stream, dtype=np.int64)
        dstl = np.full(stream, 999, dtype=np.int64)
        dst12 = np.zeros(stream, dtype=np.int64)
        key = tile_of[o] * NSEG + seg_of[o]
        # within-group position (groups consecutive after the sort)
        grp_change = np.empty(len(o), dtype=bool)
        grp_change[0] = True
        grp_change[1:] = key[1:] != key[:-1]
        gidx = np.cumsum(grp_change) - 1
        starts = np.flatnonzero(grp_change)
        within = np.arange(len(o)) - starts[gidx]
        pos = run_off[key] + within
        srcl[pos] = src_g[o] - seg_of[o] * SEGSZ
        dstl[pos] = dloc[o]
        dst12[pos] = tile_of[o] * TILE + dloc[o]

        def wrap16(a):
            blk = a.reshape(stream // 16, 16).T.astype(np.int16)
            return np.tile(blk, (8, 1))

        percore.append(dict(
            src_idx=np.ascontiguousarray(wrap16(srcl)),
            dst_idx=np.ascontiguousarray(wrap16(dst12)),
            dst_col=np.ascontiguousarray(
                dstl.reshape(nchunks, TILE).T.astype(np.float32)),
        ))

    sched = dict(nchunks=nchunks, stream=stream,
                 tile_of_chunk=tile_of_chunk, runs=runs)
    return sched, percore


def derive_weights(cfg, params):
    p = {k: np.asarray(v, dtype=np.float32) for k, v in params.items()}
    w = {}
    w['W1'] = p['gcn1_W']                                           # [3,64]
    Wg1 = p['gcn1_W']  # placeholder overwritten below
    Wg1 = p['gat1_W'].reshape(64, 4, 64)
    w['AsAd1'] = np.concatenate([
        np.einsum('khf,hf->kh', Wg1, p['gat1_as']),
        np.einsum('khf,hf->kh', Wg1, p['gat1_ad'])], axis=1)        # [64,8]
    w['Wg1'] = np.ascontiguousarray(Wg1)                            # [64,4,64] (k,h,o)
    Wg2 = p['gat2_W'].reshape(128, 2, 128)
    w['AsAd2'] = np.concatenate([
        np.einsum('khf,hf->kh', Wg2, p['gat2_as']),
        np.einsum('khf,hf->kh', Wg2, p['gat2_ad'])], axis=1)        # [128,4]
    w['Wg2'] = np.ascontiguousarray(Wg2)                            # [128,2,128]
    w['W2'] = np.ascontiguousarray(
        p['gcn2_W'].reshape(2, 128, 128).transpose(1, 0, 2))        # [128,2,128]
    w['W3'] = np.ascontiguousarray(
        p['gcn3_W'].reshape(2, 128, 256).transpose(1, 0, 2))        # [128,2,256]
    w['fc1_W'] = np.ascontiguousarray(
        p['fc1_W'].reshape(6, 128, 128).transpose(1, 0, 2))         # [128,6,128]
    w['fc2_W'] = p['fc2_W']                                         # [128,1]
    for k in ['gcn1_b', 'gat1_b', 'gcn2_b', 'gat2_b', 'gcn3_b',
              'bn1_g', 'bn1_b', 'bn2_g', 'bn2_b',
              'fc1_bias', 'fc1_g', 'fc1_beta', 'fc2_b']:
        w[k] = np.ascontiguousarray(np.tile(p[k][None, :], (128, 1)))
    return w


def build(cfg, sched, wts, debug=False):
    from concourse import bacc, mybir
    import concourse.tile as tile
    from concourse.library_config import mlp
    from concourse._compat import cdiv

    f32 = mybir.dt.float32
    i16 = mybir.dt.int16
    Alu = mybir.AluOpType
    Act = mybir.ActivationFunctionType
    AX = mybir.AxisListType

    TILE, NT, WIN = cfg.TILE, cfg.NTILES, cfg.WIN
    NCH, STREAM = sched['nchunks'], sched['stream']
    NWIN = cdiv(NCH, WIN)
    toc = sched['tile_of_chunk']
    runs = sched['runs']
    SHP, SEGSZ, NSEG = cfg.SHARD_P, cfg.SEGSZ, cfg.NSEG
    NPAD = cfg.NPAD

    nc = bacc.Bacc("TRN2", target_bir_lowering=False, debug=False,
                   num_devices=cfg.NC, num_swdge_queues=4)

    def din(name, shape, dt=f32):
        return nc.dram_tensor(name, list(shape), dt, kind="ExternalInput")

    xT_d = din("xT", [3, SHP])
    src_idx = din("src_idx", [128, STREAM // 16], i16)
    dst_idx = din("dst_idx", [128, STREAM // 16], i16)
    dst_col_d = din("dst_col", [128, NCH])
    batch_col_d = din("batch_col", [128, NT])
    mask_col_d = din("mask_col", [128, NT])
    iota_d = din("iota128", [128, 128])
    iota8_d = din("iota8", [128, 8])
    ident_d = din("ident", [128, 128])
    wd = {k: din("w_" + k, v.shape) for k, v in wts.items()}
    out_d = nc.dram_tensor("out", [cfg.G, 1], f32, kind="ExternalOutput")
    dbg = {}
    if debug:
        for nm, shape in [("dbg_deg", [128, (cfg.SHARD_P + 127) // 128]),
                          ("dbg_t1", [cfg.SHARD_P, 128]),
                          ("dbg_y1", [cfg.SHARD_P, 128]),
                          ("dbg_t2", [cfg.SHARD_P, 192]),
                          ("dbg_y2", [cfg.SHARD_P, 256]),
                          ("dbg_g3", [cfg.SHARD_P, 256]),
                          ("dbg_pool", [8, 772]),
                          ("dbg_aldc2", [cfg.SHARD_P, 64]),
                          ("dbg_mx2", [1, 2])]:
            dbg[nm] = nc.dram_tensor(nm, shape, f32, kind="ExternalOutput")

    def dram(name, shape, shared=False):
        return nc.dram_tensor(name, list(shape), f32,
                              addr_space="Shared" if shared else "Local")

    t0_l = dram("t0_l", [SHP, 64]);  t0_f = dram("t0_f", [NPAD, 64], True)
    t1_l = dram("t1_l", [SHP, 128]); t1_f = dram("t1_f", [NPAD, 128], True)
    y1_l = dram("y1_l", [SHP, 128]); y1_f = dram("y1_f", [NPAD, 128], True)
    t2_l = dram("t2_l", [SHP, 192]); t2_f = dram("t2_f", [NPAD, 192], True)
    y2_l = dram("y2_l", [SHP, 256]); y2_f = dram("y2_f", [NPAD, 256], True)
    aldc1_d = dram("aldc1", [SHP, 64])
    aldc2_d = dram("aldc2", [SHP, 64])
    x1_d = dram("x1s", [SHP, 256])
    x2_d = dram("x2s", [SHP, 256])
    g2_d = dram("g2s", [SHP, 128])
    g3_d = dram("g3s", [SHP, 256])
    mx1_l = dram("mx1_l", [1, 4]);  mx1_f = dram("mx1_f", [1, 4], True)
    mx2_l = dram("mx2_l", [1, 2]);  mx2_f = dram("mx2_f", [1, 2], True)
    bn1_l = dram("bn1_l", [1, 256]); bn1_f = dram("bn1_f", [1, 256], True)
    bn2_l = dram("bn2_l", [1, 512]); bn2_f = dram("bn2_f", [1, 512], True)
    pool_l = dram("pool_l", [8, 772]); pool_f = dram("pool_f", [8, 772], True)

    RG = [list(range(cfg.NC))]

    runs_by_win = {}
    for (off, n, s) in runs:
        runs_by_win.setdefault(off // (WIN * 128), []).append((off, n, s))

    with tile.TileContext(nc) as tc, ExitStack() as ex:
        nc.gpsimd.load_library(mlp)
        res = ex.enter_context(tc.tile_pool(name="res", bufs=1))
        sb = ex.enter_context(tc.tile_pool(name="sb", bufs=2))
        gwp = ex.enter_context(tc.tile_pool(name="gw", bufs=2))
        pp = ex.enter_context(tc.tile_pool(name="pp", bufs=2, space="PSUM"))
        ppagg = ex.enter_context(tc.tile_pool(name="ppagg", bufs=2, space="PSUM"))

        def load(dr, shape, dt=f32):
            t = res.tile(list(shape), dt, tag=dr.name)
            nc.sync.dma_start(out=t[:], in_=dr[:])
            return t

        src_i = load(src_idx, [128, STREAM // 16], i16)
        dst_i = load(dst_idx, [128, STREAM // 16], i16)
        dstc = load(dst_col_d, [128, NCH])
        batc = load(batch_col_d, [128, NT])
        maskc = load(mask_col_d, [128, NT])
        iota = load(iota_d, [128, 128])
        iota8 = load(iota8_d, [128, 8])
        ident = load(ident_d, [128, 128])
        W = {k: load(wd[k], wts[k].shape) for k in wts}

        dinv = res.tile([128, NT], f32, tag="dinv")
        alsd1 = res.tile([128, NT * 8], f32, tag="alsd1")
        alsd2 = res.tile([128, NT * 4], f32, tag="alsd2")
        ones = res.tile([128, 4], f32, tag="ones")
        nc.gpsimd.memset(ones[:], 1.0)

        qn = [0]

        # ---------- helpers ----------
        def onehot_win(w):
            c0 = w * WIN
            ncc = min(WIN, NCH - c0)
            P = sb.tile([128, WIN, 128], f32, tag="P")
            nc.vector.tensor_tensor(
                out=P[:, :ncc, :],
                in0=dstc[:, c0:c0 + ncc].unsqueeze(2).to_broadcast([128, ncc, 128]),
                in1=iota[:, :].unsqueeze(1).to_broadcast([128, ncc, 128]),
                op=Alu.is_equal)
            return P

        def gather_src_win(w, table_f, width, col0=0, tabw=None):
            g = gwp.tile([128, WIN, width], f32, tag="gw")
            e0 = w * WIN * 128
            for (off, n, s) in runs_by_win.get(w, []):
                col = (off - e0) // 128
                kc = n // 128
                base = s * SEGSZ
                hi = min(base + SEGSZ, NPAD)
                src_ap = table_f[base:hi, col0:col0 + width] if col0 else \
                    table_f[base:hi, :]
                nc.gpsimd.dma_gather(
                    g[:, col:col + kc, :], src_ap,
                    src_i[:, off // 16:(off + n) // 16],
                    n, n, width,
                    elem_step=(tabw if tabw else width),
                    queue_num=qn[0] % 4)
                qn[0] += 1
            return g

        def gather_dst_win(w, table):
            g = gwp.tile([128, WIN, 64], f32, tag="gd")
            e0 = w * WIN * 128
            e1 = min((w + 1) * WIN * 128, STREAM)
            n = e1 - e0
            nc.gpsimd.dma_gather(
                g[:, :n // 128, :], table[:],
                dst_i[:, e0 // 16:e1 // 16],
                n, n, 64, queue_num=qn[0] % 4)
            qn[0] += 1
            return g

        def agg_pass(table_f, width, rhs_fn, post_fn, rw, col0=0, tabw=None):
            cur = {}
            for w in range(NWIN):
                c0, c1 = w * WIN, min((w + 1) * WIN, NCH)
                g = gather_src_win(w, table_f, width, col0, tabw) \
                    if table_f is not None else None
                P = onehot_win(w)
                slicer = rhs_fn(w, g)
                for c in range(c0, c1):
                    t = int(toc[c])
                    first = (c == 0) or (int(toc[c - 1]) != t)
                    last = (c == NCH - 1) or (int(toc[c + 1]) != t)
                    if first:
                        cur[t] = ppagg.tile([128, rw], f32, tag="agg", name=f"agg{t}", space="PSUM")
                    nc.tensor.matmul(
                        out=cur[t][:, :rw],
                        lhsT=P[:, c - c0, :],
                        rhs=slicer(c - c0),
                        start=first, stop=last)
                    if last:
                        post_fn(t, cur.pop(t))

        def transpose_to_sbuf(src_ap, n_par, n_free, tag):
            ps = pp.tile([n_free, n_par], f32, tag="tp", space="PSUM")
            nc.tensor.transpose(out=ps[:, :n_par], in_=src_ap,
                                identity=ident[:n_par, :n_par])
            s = sb.tile([n_free, n_par], f32, tag=tag)
            nc.vector.tensor_copy(out=s[:, :n_par], in_=ps[:, :n_par])
            return s

        def row_add(dst_ap, in_ap, row_ap):
            nc.vector.tensor_tensor(out=dst_ap, in0=in_ap, in1=row_ap, op=Alu.add)

        def lrelu_(out_ap, in_ap, tmp_ap):
            nc.vector.tensor_scalar_mul(tmp_ap, in_ap, 0.2)
            nc.vector.tensor_tensor(out=out_ap, in0=in_ap, in1=tmp_ap, op=Alu.max)

        ones1 = res.tile([1, 128], f32, tag="ones1")
        nc.gpsimd.memset(ones1[:], 1.0)
        invmc = res.tile([128, NT], f32, tag="invmc")
        nc.vector.tensor_scalar_mul(invmc[:], maskc[:], -1.0)
        nc.vector.tensor_scalar_add(invmc[:], invmc[:], 1.0)

        def replicate_row(row_ap, width, tag, npar=128):
            ps = pp.tile([128, 256], f32, tag="mm", space="PSUM")
            nc.tensor.matmul(out=ps[:npar, :width], lhsT=ones1[:, :npar],
                             rhs=row_ap, start=True, stop=True)
            t = res.tile([128, width], f32, tag=tag, name=tag)
            nc.vector.tensor_copy(out=t[:npar, :width], in_=ps[:npar, :width])
            return t

        # ================= P0: deg =================
        deg = res.tile([128, NT], f32, tag="deg")

        def post_deg(t, ps):
            nc.vector.tensor_copy(out=deg[:, t:t + 1], in_=ps[:, :1])

        agg_pass(None, 0, lambda w, g: (lambda lc: ones[:, :1]), post_deg, 1)

        tmp = sb.tile([128, NT], f32, tag="dtmp")
        nc.vector.tensor_tensor(out=tmp[:], in0=deg[:], in1=maskc[:], op=Alu.subtract)
        nc.vector.tensor_scalar_add(tmp[:], tmp[:], 1.0)   # deg - mask + 1
        rec = sb.tile([128, NT], f32, tag="drec")
        nc.vector.reciprocal(out=rec[:], in_=tmp[:])
        nc.scalar.activation(dinv[:], rec[:], Act.Sqrt)
        nc.vector.tensor_tensor(out=dinv[:], in0=dinv[:], in1=maskc[:], op=Alu.mult)

        # ================= P1: t0 = dinv * (x @ W1) =================
        for t in range(NT):
            xTt = sb.tile([3, 128], f32, tag="xTt", name="xTt")
            nc.sync.dma_start(out=xTt[:], in_=xT_d[:, t * 128:(t + 1) * 128])
            ps = pp.tile([128, 256], f32, tag="mm", space="PSUM")
            nc.tensor.matmul(out=ps[:, :64], lhsT=xTt[:],
                             rhs=W['W1'][:], start=True, stop=True)
            s = sb.tile([128, 64], f32, tag="t0t")
            nc.scalar.activation(s[:], ps[:, :64], Act.Copy, scale=dinv[:, t:t + 1])
            nc.sync.dma_start(out=t0_l[t * 128:(t + 1) * 128, :], in_=s[:])
        nc.gpsimd.collective_compute("AllGather", Alu.bypass, replica_groups=RG,
                                     ins=[t0_l[:]], outs=[t0_f[:]])

        # ================= P2: gcn1 =================
        mxacc1 = res.tile([128, 4], f32, tag="mx1")
        nc.gpsimd.memset(mxacc1[:], -1e30)

        def post_gcn1(t, ps):
            x1a = sb.tile([128, 64], f32, tag="x1a")
            nc.scalar.activation(x1a[:], ps[:, :64], Act.Copy, scale=dinv[:, t:t + 1])
            row_add(x1a[:], x1a[:], W['gcn1_b'][:, :64])
            nc.scalar.activation(x1a[:], x1a[:], Act.Relu)
            x1aT = transpose_to_sbuf(x1a[:], 128, 64, "x1aT")
            ps2 = pp.tile([128, 8], f32, tag="sm", space="PSUM")
            nc.tensor.matmul(out=ps2[:], lhsT=x1aT[:, :128], rhs=W['AsAd1'][:],
                             start=True, stop=True)
            nc.vector.tensor_copy(out=alsd1[:, t * 8:(t + 1) * 8], in_=ps2[:])
            nc.vector.tensor_tensor(out=mxacc1[:], in0=mxacc1[:], in1=ps2[:, 0:4],
                                    op=Alu.max)
            nc.sync.dma_start(out=t1_l[t * 128:(t + 1) * 128, 0:64], in_=x1a[:])
            nc.sync.dma_start(out=t1_l[t * 128:(t + 1) * 128, 64:72],
                              in_=alsd1[:, t * 8:(t + 1) * 8])

        agg_pass(t0_f, 64, lambda w, g: (lambda lc: g[:, lc, :64]), post_gcn1, 64)

        mxT = transpose_to_sbuf(mxacc1[:], 128, 4, "mxT")      # [4,128]
        mx = sb.tile([4, 1], f32, tag="mxr")
        nc.vector.tensor_reduce(out=mx[:], in_=mxT[:, :128], axis=AX.X, op=Alu.max)
        mxrT = transpose_to_sbuf(mx[:], 4, 1, "mxrT")          # [1,4]
        nc.sync.dma_start(out=mx1_l[:], in_=mxrT[:, :4])
        nc.gpsimd.collective_compute("AllReduce", Alu.max, replica_groups=RG,
                                     ins=[mx1_l[:]], outs=[mx1_f[:]])
        mxs1 = sb.tile([1, 4], f32, tag="mxs1")
        nc.sync.dma_start(out=mxs1[:], in_=mx1_f[:])
        mxs1r = replicate_row(mxs1[:1, :4], 4, "mxs1r")

        for t in range(NT):
            a = sb.tile([128, 8], f32, tag="aldc")
            nc.vector.tensor_copy(out=a[:, 0:4], in_=alsd1[:, t * 8 + 4:t * 8 + 8])
            row_add(a[:, 4:8], a[:, 0:4], mxs1r[:, :4])
            at = sb.tile([128, 8], f32, tag="aldt", name="aldt")
            lrelu_(a[:, 4:8], a[:, 4:8], at[:, 0:4])
            nc.sync.dma_start(out=aldc1_d[t * 128:(t + 1) * 128, 0:8], in_=a[:])
        nc.gpsimd.collective_compute("AllGather", Alu.bypass, replica_groups=RG,
                                     ins=[t1_l[:]], outs=[t1_f[:]])

        # ================= P3: gat1 =================
        def rhs_gat1(w, g):
            c0 = w * WIN
            ncc = min(WIN, NCH - c0)
            ald = gather_dst_win(w, aldc1_d)
            ex = sb.tile([128, WIN, 4], f32, tag="ex")
            nc.vector.tensor_tensor(out=ex[:, :ncc, :], in0=g[:, :ncc, 64:68],
                                    in1=ald[:, :ncc, 0:4], op=Alu.add)
            ext = sb.tile([128, WIN, 4], f32, tag="ext", name="ext")
            lrelu_(ex[:, :ncc, :], ex[:, :ncc, :], ext[:, :ncc, :])
            nc.vector.tensor_tensor(out=ex[:, :ncc, :], in0=ex[:, :ncc, :],
                                    in1=ald[:, :ncc, 4:8], op=Alu.subtract)
            nc.vector.tensor_scalar_max(ex[:, :ncc, :], ex[:, :ncc, :], -80.0)
            nc.scalar.activation(ex[:, :ncc, :], ex[:, :ncc, :], Act.Exp)
            rhs = sb.tile([128, WIN, 260], f32, tag="rhs", bufs=1)
            for h in range(4):
                nc.vector.tensor_tensor(
                    out=rhs[:, :ncc, h * 64:(h + 1) * 64],
                    in0=g[:, :ncc, 0:64],
                    in1=ex[:, :ncc, h:h + 1].to_broadcast([128, ncc, 64]),
                    op=Alu.mult)
            nc.vector.tensor_copy(out=rhs[:, :ncc, 256:260], in_=ex[:, :ncc, :])
            return lambda lc: rhs[:, lc, :260]

        def post_gat1(t, ps):
            den = sb.tile([128, 4], f32, tag="den")
            nc.vector.tensor_tensor(
                out=den[:], in0=ps[:, 256:260],
                in1=invmc[:, t:t + 1].to_broadcast([128, 4]), op=Alu.add)
            rcp = sb.tile([128, 4], f32, tag="rcp")
            nc.vector.reciprocal(out=rcp[:], in_=den[:])
            psx = pp.tile([128, 256], f32, tag="mm", space="PSUM")
            for h in range(4):
                nh = sb.tile([128, 64], f32, tag="nh")
                nc.scalar.activation(nh[:], ps[:, h * 64:(h + 1) * 64], Act.Copy,
                                     scale=rcp[:, h:h + 1])
                nhT = transpose_to_sbuf(nh[:], 128, 64, "nhT")
                nc.tensor.matmul(out=psx[:, h * 64:(h + 1) * 64], lhsT=nhT[:, :128],
                                 rhs=W['Wg1'][:, h, :], start=True, stop=True)
            x1 = sb.tile([128, 256], f32, tag="x1t")
            row_add(x1[:], psx[:, :256], W['gat1_b'][:, :256])
            nc.sync.dma_start(out=x1_d[t * 128:(t + 1) * 128, :], in_=x1[:])
            x1T0 = transpose_to_sbuf(x1[:, 0:128], 128, 128, "xT0")
            x1T1 = transpose_to_sbuf(x1[:, 128:256], 128, 128, "xT1")
            psy = pp.tile([128, 128], f32, tag="mm", space="PSUM")
            nc.tensor.matmul(out=psy[:], lhsT=x1T0[:, :128], rhs=W['W2'][:, 0, :],
                             start=True, stop=False)
            nc.tensor.matmul(out=psy[:], lhsT=x1T1[:, :128], rhs=W['W2'][:, 1, :],
                             start=False, stop=True)
            y1 = sb.tile([128, 128], f32, tag="y1t")
            nc.scalar.activation(y1[:], psy[:], Act.Copy, scale=dinv[:, t:t + 1])
            nc.sync.dma_start(out=y1_l[t * 128:(t + 1) * 128, :], in_=y1[:])

        agg_pass(t1_f, 128, rhs_gat1, post_gat1, 260)
        nc.gpsimd.collective_compute("AllGather", Alu.bypass, replica_groups=RG,
                                     ins=[y1_l[:]], outs=[y1_f[:]])

        # ================= P4: gcn2 + BN1 =================
        bnacc = res.tile([1, 256], f32, tag="bnacc")
        nc.gpsimd.memset(bnacc[:], 0.0)

        def post_gcn2(t, ps):
            g2 = sb.tile([128, 128], f32, tag="g2t")
            nc.scalar.activation(g2[:], ps[:, :128], Act.Copy, scale=dinv[:, t:t + 1])
            row_add(g2[:], g2[:], W['gcn2_b'][:, :128])
            nc.sync.dma_start(out=g2_d[t * 128:(t + 1) * 128, :], in_=g2[:])
            sq = sb.tile([128, 128], f32, tag="g2sq")
            nc.vector.tensor_tensor(out=sq[:], in0=g2[:], in1=g2[:], op=Alu.mult)
            pba = pp.tile([1, 128], f32, tag="sm", space="PSUM")
            pbb = pp.tile([1, 128], f32, tag="sm", space="PSUM")
            nc.tensor.matmul(out=pba[:], lhsT=maskc[:, t:t + 1], rhs=g2[:],
                             start=True, stop=True)
            nc.tensor.matmul(out=pbb[:], lhsT=maskc[:, t:t + 1], rhs=sq[:],
                             start=True, stop=True)
            nc.vector.tensor_tensor(out=bnacc[:, 0:128], in0=bnacc[:, 0:128],
                                    in1=pba[:, :128], op=Alu.add)
            nc.vector.tensor_tensor(out=bnacc[:, 128:256], in0=bnacc[:, 128:256],
                                    in1=pbb[:, :128], op=Alu.add)

        agg_pass(y1_f, 128, lambda w, g: (lambda lc: g[:, lc, :128]), post_gcn2, 128)
        nc.sync.dma_start(out=bn1_l[:], in_=bnacc[:])
        nc.gpsimd.collective_compute("AllReduce", Alu.add, replica_groups=RG,
                                     ins=[bn1_l[:]], outs=[bn1_f[:]])
        bnf = sb.tile([1, 256], f32, tag="bnf")
        nc.sync.dma_start(out=bnf[:], in_=bn1_f[:])
        bn_a1 = res.tile([1, 128], f32, tag="bn_a1")
        bn_b1 = res.tile([1, 128], f32, tag="bn_b1")

        def bn_coeffs(bnf_t, g_ap, beta_ap, a_out, b_out, width, nn):
            mu = sb.tile([1, 256], f32, tag="bmu")
            nc.vector.tensor_scalar_mul(mu[:, :width], bnf_t[:1, 0:width], 1.0 / nn)
            nc.vector.tensor_scalar_mul(mu[:, width:2 * width],
                                        bnf_t[:1, width:2 * width], 1.0 / nn)
            v = sb.tile([1, 256], f32, tag="bv", name="bv")
            nc.vector.tensor_tensor(out=v[:, :width], in0=mu[:, :width],
                                    in1=mu[:, :width], op=Alu.mult)
            nc.vector.tensor_tensor(out=v[:, :width], in0=mu[:, width:2 * width],
                                    in1=v[:, :width], op=Alu.subtract)
            nc.vector.tensor_scalar_add(v[:, :width], v[:, :width], 1e-5)
            nc.vector.reciprocal(out=v[:, :width], in_=v[:, :width])
            nc.scalar.activation(v[:, :width], v[:, :width], Act.Sqrt)
            nc.vector.tensor_tensor(out=a_out[:, :width], in0=g_ap, in1=v[:, :width],
                                    op=Alu.mult)
            nc.vector.tensor_tensor(out=b_out[:, :width], in0=mu[:, :width],
                                    in1=a_out[:, :width], op=Alu.mult)
            nc.vector.tensor_tensor(out=b_out[:, :width], in0=beta_ap,
                                    in1=b_out[:, :width], op=Alu.subtract)

        bn_coeffs(bnf, W['bn1_g'][:1, :128], W['bn1_b'][:1, :128], bn_a1, bn_b1, 128, cfg.N)
        bn_a1r = replicate_row(bn_a1[:1, :128], 128, "bn_a1r")
        bn_b1r = replicate_row(bn_b1[:1, :128], 128, "bn_b1r")

        mxacc2 = res.tile([128, 2], f32, tag="mx2")
        nc.gpsimd.memset(mxacc2[:], -1e30)
        for t in range(NT):
            g2 = sb.tile([128, 128], f32, tag="g2r")
            nc.sync.dma_start(out=g2[:], in_=g2_d[t * 128:(t + 1) * 128, :])
            x2a = sb.tile([128, 128], f32, tag="x2a")
            nc.vector.tensor_tensor(out=x2a[:], in0=g2[:],
                                    in1=bn_a1r[:, :128], op=Alu.mult)
            row_add(x2a[:], x2a[:], bn_b1r[:, :128])
            nc.scalar.activation(x2a[:], x2a[:], Act.Relu)
            x2aT = transpose_to_sbuf(x2a[:], 128, 128, "x2aT")
            ps2 = pp.tile([128, 8], f32, tag="sm", space="PSUM")
            nc.tensor.matmul(out=ps2[:, :4], lhsT=x2aT[:, :128], rhs=W['AsAd2'][:],
                             start=True, stop=True)
            nc.vector.tensor_copy(out=alsd2[:, t * 4:(t + 1) * 4], in_=ps2[:, :4])
            nc.vector.tensor_tensor(out=mxacc2[:], in0=mxacc2[:], in1=ps2[:, 0:2],
                                    op=Alu.max)
            nc.sync.dma_start(out=t2_l[t * 128:(t + 1) * 128, 0:128], in_=x2a[:])
            nc.sync.dma_start(out=t2_l[t * 128:(t + 1) * 128, 128:132],
                              in_=alsd2[:, t * 4:(t + 1) * 4])
        mxT2 = transpose_to_sbuf(mxacc2[:], 128, 2, "mxT2")
        mx2 = sb.tile([2, 1], f32, tag="mx2r")
        nc.vector.tensor_reduce(out=mx2[:], in_=mxT2[:, :128], axis=AX.X, op=Alu.max)
        mxr2T = transpose_to_sbuf(mx2[:], 2, 1, "mxr2T")
        nc.sync.dma_start(out=mx2_l[:], in_=mxr2T[:, :2])
        nc.gpsimd.collective_compute("AllReduce", Alu.max, replica_groups=RG,
                                     ins=[mx2_l[:]], outs=[mx2_f[:]])
        mxs2 = sb.tile([1, 2], f32, tag="mxs2")
        nc.sync.dma_start(out=mxs2[:], in_=mx2_f[:])
        mxs2r = replicate_row(mxs2[:1, :2], 2, "mxs2r")
        for t in range(NT):
            a = sb.tile([128, 4], f32, tag="aldc2")
            nc.vector.tensor_copy(out=a[:, 0:2], in_=alsd2[:, t * 4 + 2:t * 4 + 4])
            row_add(a[:, 2:4], a[:, 0:2], mxs2r[:, :2])
            at = sb.tile([128, 8], f32, tag="aldt", name="aldt")
            lrelu_(a[:, 2:4], a[:, 2:4], at[:, 0:2])
            nc.sync.dma_start(out=aldc2_d[t * 128:(t + 1) * 128, 0:4], in_=a[:])
        nc.gpsimd.collective_compute("AllGather", Alu.bypass, replica_groups=RG,
                                     ins=[t2_l[:]], outs=[t2_f[:]])

        # ================= P5a: gat2 den pre-pass (shift refinement) ====
        def rhs_denA(w, g):
            c0 = w * WIN
            ncc = min(WIN, NCH - c0)
            ald = gather_dst_win(w, aldc2_d)
            exa = sb.tile([128, WIN, 2], f32, tag="ex2", name="exa")
            nc.vector.tensor_tensor(out=exa[:, :ncc, :], in0=g[:, :ncc, 0:2],
                                    in1=ald[:, :ncc, 0:2], op=Alu.add)
            ext = sb.tile([128, WIN, 2], f32, tag="ext2", name="extA")
            lrelu_(exa[:, :ncc, :], exa[:, :ncc, :], ext[:, :ncc, :])
            nc.vector.tensor_tensor(out=exa[:, :ncc, :], in0=exa[:, :ncc, :],
                                    in1=ald[:, :ncc, 2:4], op=Alu.subtract)
            nc.vector.tensor_scalar_max(exa[:, :ncc, :], exa[:, :ncc, :], -80.0)
            nc.scalar.activation(exa[:, :ncc, :], exa[:, :ncc, :], Act.Exp)
            return lambda lc: exa[:, lc, :]

        def post_denA(t, ps):
            dA = sb.tile([128, 2], f32, tag="dA", name="dA")
            nc.vector.tensor_tensor(
                out=dA[:], in0=ps[:, :2],
                in1=invmc[:, t:t + 1].to_broadcast([128, 2]), op=Alu.add)
            nc.scalar.activation(dA[:], dA[:], Act.Ln)
            a = sb.tile([128, 4], f32, tag="aldc2", name="aldc2b")
            nc.vector.tensor_copy(out=a[:, 0:2], in_=alsd2[:, t * 4 + 2:t * 4 + 4])
            row_add(a[:, 2:4], a[:, 0:2], mxs2r[:, :2])
            at = sb.tile([128, 8], f32, tag="aldt", name="aldtA")
            lrelu_(a[:, 2:4], a[:, 2:4], at[:, 0:2])
            nc.vector.tensor_tensor(out=a[:, 2:4], in0=a[:, 2:4], in1=dA[:],
                                    op=Alu.add)
            nc.sync.dma_start(out=aldc2_d[t * 128:(t + 1) * 128, 0:4], in_=a[:])

        agg_pass(t2_f, 64, rhs_denA, post_denA, 2, col0=128, tabw=192)

        # ================= P5: gat2 =================
        def rhs_gat2(w, g):
            c0 = w * WIN
            ncc = min(WIN, NCH - c0)
            ald = gather_dst_win(w, aldc2_d)
            ex = sb.tile([128, WIN, 2], f32, tag="ex2", name="ex2m")
            nc.vector.tensor_tensor(out=ex[:, :ncc, :], in0=g[:, :ncc, 128:130],
                                    in1=ald[:, :ncc, 0:2], op=Alu.add)
            ext = sb.tile([128, WIN, 2], f32, tag="ext2", name="ext2m")
            lrelu_(ex[:, :ncc, :], ex[:, :ncc, :], ext[:, :ncc, :])
            nc.vector.tensor_tensor(out=ex[:, :ncc, :], in0=ex[:, :ncc, :],
                                    in1=ald[:, :ncc, 2:4], op=Alu.subtract)
            nc.vector.tensor_scalar_max(ex[:, :ncc, :], ex[:, :ncc, :], -80.0)
            nc.scalar.activation(ex[:, :ncc, :], ex[:, :ncc, :], Act.Exp)
            rhs = sb.tile([128, WIN, 260], f32, tag="rhs", bufs=1)
            for h in range(2):
                nc.vector.tensor_tensor(
                    out=rhs[:, :ncc, h * 128:(h + 1) * 128],
                    in0=g[:, :ncc, 0:128],
                    in1=ex[:, :ncc, h:h + 1].to_broadcast([128, ncc, 128]),
                    op=Alu.mult)
            nc.vector.tensor_copy(out=rhs[:, :ncc, 256:258], in_=ex[:, :ncc, :])
            return lambda lc: rhs[:, lc, :258]

        def post_gat2(t, ps):
            den = sb.tile([128, 2], f32, tag="den")
            nc.vector.tensor_tensor(
                out=den[:, :2], in0=ps[:, 256:258],
                in1=invmc[:, t:t + 1].to_broadcast([128, 2]), op=Alu.add)
            rcp = sb.tile([128, 2], f32, tag="rcp")
            nc.vector.reciprocal(out=rcp[:, :2], in_=den[:, :2])
            psx = pp.tile([128, 256], f32, tag="mm", space="PSUM")
            for h in range(2):
                nh = sb.tile([128, 128], f32, tag="nh")
                nc.scalar.activation(nh[:, :128], ps[:, h * 128:(h + 1) * 128],
                                     Act.Copy, scale=rcp[:, h:h + 1])
                nhT = transpose_to_sbuf(nh[:, :128], 128, 128, "nhT")
                nc.tensor.matmul(out=psx[:, h * 128:(h + 1) * 128], lhsT=nhT[:, :128],
                                 rhs=W['Wg2'][:, h, :], start=True, stop=True)
            x2 = sb.tile([128, 256], f32, tag="x2t")
            row_add(x2[:], psx[:, :256], W['gat2_b'][:, :256])
            nc.sync.dma_start(out=x2_d[t * 128:(t + 1) * 128, :], in_=x2[:])
            y2 = sb.tile([128, 256], f32, tag="y2t")
            nc.scalar.activation(y2[:], x2[:], Act.Copy, scale=dinv[:, t:t + 1])
            nc.sync.dma_start(out=y2_l[t * 128:(t + 1) * 128, :], in_=y2[:])

        agg_pass(t2_f, 192, rhs_gat2, post_gat2, 258)
        nc.gpsimd.collective_compute("AllGather", Alu.bypass, replica_groups=RG,
                                     ins=[y2_l[:]], outs=[y2_f[:]])

        # ================= P6: gcn3 + BN2 =================
        bnacc2 = res.tile([1, 512], f32, tag="bnacc2")
        nc.gpsimd.memset(bnacc2[:], 0.0)

        def post_gcn3(t, ps):
            ag = sb.tile([128, 256], f32, tag="ag3")
            nc.scalar.activation(ag[:], ps[:, :256], Act.Copy, scale=dinv[:, t:t + 1])
            agT0 = transpose_to_sbuf(ag[:, 0:128], 128, 128, "xT0")
            agT1 = transpose_to_sbuf(ag[:, 128:256], 128, 128, "xT1")
            psg = pp.tile([128, 256], f32, tag="mm", space="PSUM")
            nc.tensor.matmul(out=psg[:, :256], lhsT=agT0[:, :128], rhs=W['W3'][:, 0, :],
                             start=True, stop=False)
            nc.tensor.matmul(out=psg[:, :256], lhsT=agT1[:, :128], rhs=W['W3'][:, 1, :],
                             start=False, stop=True)
            g3 = sb.tile([128, 256], f32, tag="g3t")
            row_add(g3[:], psg[:, :256], W['gcn3_b'][:, :256])
            nc.sync.dma_start(out=g3_d[t * 128:(t + 1) * 128, :], in_=g3[:])
            sq = sb.tile([128, 256], f32, tag="g3sq")
            nc.vector.tensor_tensor(out=sq[:], in0=g3[:], in1=g3[:], op=Alu.mult)
            pba = pp.tile([1, 256], f32, tag="sm", space="PSUM")
            pbb = pp.tile([1, 256], f32, tag="sm", space="PSUM")
            nc.tensor.matmul(out=pba[:], lhsT=maskc[:, t:t + 1], rhs=g3[:],
                             start=True, stop=True)
            nc.tensor.matmul(out=pbb[:], lhsT=maskc[:, t:t + 1], rhs=sq[:],
                             start=True, stop=True)
            nc.vector.tensor_tensor(out=bnacc2[:, 0:256], in0=bnacc2[:, 0:256],
                                    in1=pba[:, :256], op=Alu.add)
            nc.vector.tensor_tensor(out=bnacc2[:, 256:512], in0=bnacc2[:, 256:512],
                                    in1=pbb[:, :256], op=Alu.add)

        agg_pass(y2_f, 256, lambda w, g: (lambda lc: g[:, lc, :256]), post_gcn3, 256)
        nc.sync.dma_start(out=bn2_l[:], in_=bnacc2[:])
        nc.gpsimd.collective_compute("AllReduce", Alu.add, replica_groups=RG,
                                     ins=[bn2_l[:]], outs=[bn2_f[:]])
        bnf2 = sb.tile([1, 512], f32, tag="bnf2")
        nc.sync.dma_start(out=bnf2[:], in_=bn2_f[:])
        bn_a2 = res.tile([1, 256], f32, tag="bn_a2")
        bn_b2 = res.tile([1, 256], f32, tag="bn_b2")

        def bn_coeffs2(bnf_t, g_ap, beta_ap, a_out, b_out, nn):
            mu = sb.tile([1, 512], f32, tag="bmu2")
            nc.vector.tensor_scalar_mul(mu[:, :512], bnf_t[:1, :512], 1.0 / nn)
            v = sb.tile([1, 256], f32, tag="bv3")
            nc.vector.tensor_tensor(out=v[:], in0=mu[:, :256], in1=mu[:, :256],
                                    op=Alu.mult)
            nc.vector.tensor_tensor(out=v[:], in0=mu[:, 256:512], in1=v[:],
                                    op=Alu.subtract)
            nc.vector.tensor_scalar_add(v[:], v[:], 1e-5)
            nc.vector.reciprocal(out=v[:], in_=v[:])
            nc.scalar.activation(v[:], v[:], Act.Sqrt)
            nc.vector.tensor_tensor(out=a_out[:], in0=g_ap, in1=v[:], op=Alu.mult)
            nc.vector.tensor_tensor(out=b_out[:], in0=mu[:, :256], in1=a_out[:],
                                    op=Alu.mult)
            nc.vector.tensor_tensor(out=b_out[:], in0=beta_ap, in1=b_out[:],
                                    op=Alu.subtract)

        bn_coeffs2(bnf2, W['bn2_g'][:1, :256], W['bn2_b'][:1, :256], bn_a2, bn_b2, cfg.N)
        bn_a2r = replicate_row(bn_a2[:1, :256], 256, "bn_a2r")
        bn_b2r = replicate_row(bn_b2[:1, :256], 256, "bn_b2r")

        # ================= P7: pool =================
        acc1 = res.tile([8, 256], f32, tag="acc1")
        acc2 = res.tile([8, 256], f32, tag="acc2")
        acc3 = res.tile([8, 256], f32, tag="acc3")
        accc = res.tile([8, 4], f32, tag="accc")
        for a in (acc1, acc2, acc3, accc):
            nc.gpsimd.memset(a[:], 0.0)
        for t in range(NT):
            B = sb.tile([128, 8], f32, tag="B")
            nc.vector.tensor_tensor(
                out=B[:], in0=batc[:, t:t + 1].to_broadcast([128, 8]),
                in1=iota8[:], op=Alu.is_equal)
            x3 = sb.tile([128, 256], f32, tag="x3t")
            nc.sync.dma_start(out=x3[:], in_=g3_d[t * 128:(t + 1) * 128, :])
            nc.vector.tensor_tensor(out=x3[:], in0=x3[:],
                                    in1=bn_a2r[:, :256], op=Alu.mult)
            row_add(x3[:], x3[:], bn_b2r[:, :256])
            nc.scalar.activation(x3[:], x3[:], Act.Relu)
            x1 = sb.tile([128, 256], f32, tag="px1")
            nc.sync.dma_start(out=x1[:], in_=x1_d[t * 128:(t + 1) * 128, :])
            x2 = sb.tile([128, 256], f32, tag="px2")
            nc.sync.dma_start(out=x2[:], in_=x2_d[t * 128:(t + 1) * 128, :])
            for acc, xx in ((acc1, x1), (acc2, x2), (acc3, x3)):
                pps = pp.tile([8, 256], f32, tag="mm", space="PSUM")
                nc.tensor.matmul(out=pps[:], lhsT=B[:], rhs=xx[:],
                                 start=True, stop=True)
                nc.vector.tensor_tensor(out=acc[:], in0=acc[:], in1=pps[:, :256],
                                        op=Alu.add)
            pps = pp.tile([8, 4], f32, tag="sm", space="PSUM")
            nc.tensor.matmul(out=pps[:], lhsT=B[:], rhs=ones[:], start=True,
                             stop=True)
            nc.vector.tensor_tensor(out=accc[:], in0=accc[:], in1=pps[:, :4],
                                    op=Alu.add)
        pl = sb.tile([8, 772], f32, tag="pl")
        nc.vector.tensor_copy(out=pl[:, 0:256], in_=acc1[:])
        nc.vector.tensor_copy(out=pl[:, 256:512], in_=acc2[:])
        nc.vector.tensor_copy(out=pl[:, 512:768], in_=acc3[:])
        nc.vector.tensor_copy(out=pl[:, 768:772], in_=accc[:])
        nc.sync.dma_start(out=pool_l[:], in_=pl[:])
        nc.gpsimd.collective_compute("AllReduce", Alu.add, replica_groups=RG,
                                     ins=[pool_l[:]], outs=[pool_f[:]])
        plf = sb.tile([8, 772], f32, tag="plf")
        nc.sync.dma_start(out=plf[:], in_=pool_f[:])

        cinv = sb.tile([8, 1], f32, tag="cinv")
        nc.vector.reciprocal(out=cinv[:], in_=plf[:, 768:769])
        pooled = sb.tile([8, 768], f32, tag="pooled")
        nc.scalar.activation(pooled[:], plf[:, 0:768], Act.Copy, scale=cinv[:])

        psf = pp.tile([8, 128], f32, tag="mm", space="PSUM")
        for k in range(6):
            pT = transpose_to_sbuf(pooled[:, k * 128:(k + 1) * 128], 8, 128, "pT")
            nc.tensor.matmul(out=psf[:], lhsT=pT[:, :8], rhs=W['fc1_W'][:, k, :],
                             start=(k == 0), stop=(k == 5))
        h = sb.tile([8, 128], f32, tag="hfc")
        row_add(h[:], psf[:, :128], W['fc1_bias'][:8, :128])
        nc.scalar.activation(h[:], h[:], Act.Relu)
        ones8 = sb.tile([8, 1], f32, tag="ones8")
        nc.gpsimd.memset(ones8[:], 1.0)
        hsq = sb.tile([8, 128], f32, tag="hsq")
        nc.vector.tensor_tensor(out=hsq[:], in0=h[:], in1=h[:], op=Alu.mult)
        pba = pp.tile([1, 128], f32, tag="sm", space="PSUM")
        pbb = pp.tile([1, 128], f32, tag="sm", space="PSUM")
        nc.tensor.matmul(out=pba[:], lhsT=ones8[:], rhs=h[:], start=True, stop=True)
        nc.tensor.matmul(out=pbb[:], lhsT=ones8[:], rhs=hsq[:], start=True, stop=True)
        hb = sb.tile([1, 256], f32, tag="hbn")
        nc.vector.tensor_copy(out=hb[:, 0:128], in_=pba[:, :128])
        nc.vector.tensor_copy(out=hb[:, 128:256], in_=pbb[:, :128])
        fa = sb.tile([1, 128], f32, tag="fa")
        fb = sb.tile([1, 128], f32, tag="fb")
        bn_coeffs(hb, W['fc1_g'][:1, :128], W['fc1_beta'][:1, :128], fa, fb, 128, 8)
        far = replicate_row(fa[:1, :128], 128, "far", npar=8)
        fbr = replicate_row(fb[:1, :128], 128, "fbr", npar=8)
        nc.vector.tensor_tensor(out=h[:], in0=h[:], in1=far[:8, :128],
                                op=Alu.mult)
        row_add(h[:], h[:], fbr[:8, :128])
        hT = transpose_to_sbuf(h[:], 8, 128, "hT")
        pso = pp.tile([8, 4], f32, tag="sm", space="PSUM")
        nc.tensor.matmul(out=pso[:, :1], lhsT=hT[:, :8], rhs=W['fc2_W'][:],
                         start=True, stop=True)
        if debug:
            dgt = sb.tile([128, NT], f32, tag="dgt")
            nc.vector.tensor_copy(out=dgt[:], in_=deg[:])
            nc.sync.dma_start(out=dbg["dbg_deg"][:, :NT], in_=dgt[:])
            nc.sync.dma_start(out=dbg["dbg_t1"][:], in_=t1_l[:])
            nc.sync.dma_start(out=dbg["dbg_y1"][:], in_=y1_l[:])
            nc.sync.dma_start(out=dbg["dbg_t2"][:], in_=t2_l[:])
            nc.sync.dma_start(out=dbg["dbg_y2"][:], in_=y2_l[:])
            nc.sync.dma_start(out=dbg["dbg_g3"][:], in_=g3_d[:])
            nc.sync.dma_start(out=dbg["dbg_pool"][:], in_=pool_f[:])
            nc.sync.dma_start(out=dbg["dbg_aldc2"][:], in_=aldc2_d[:])
            nc.sync.dma_start(out=dbg["dbg_mx2"][:], in_=mx2_f[:])
        o = sb.tile([8, 1], f32, tag="oout")
        row_add(o[:], pso[:, :1], W['fc2_b'][:8, :1])
        nc.sync.dma_start(out=out_d[:], in_=o[:])

    nc.compile()
    return nc


def _host_inputs(cfg, x, batch, percore, wts):
    NT, TILE, SH, SHP = cfg.NTILES, cfg.TILE, cfg.SHARD, cfg.SHARD_P
    iota128 = np.tile(np.arange(128, dtype=np.float32), (128, 1))
    iota8 = np.tile(np.arange(8, dtype=np.float32), (128, 1))
    ident = np.eye(128, dtype=np.float32)
    ins = []
    for c in range(cfg.NC):
        sl = slice(c * SH, (c + 1) * SH)
        xs = np.zeros((SHP, 3), dtype=np.float32)
        xs[:SH] = x[sl]
        bc = np.full(SHP, 999.0, dtype=np.float32)
        bc[:SH] = batch[sl].astype(np.float32)
        mk = np.zeros(SHP, dtype=np.float32)
        mk[:SH] = 1.0
        m = dict(
            xT=np.ascontiguousarray(xs.T),
            src_idx=percore[c]['src_idx'],
            dst_idx=percore[c]['dst_idx'],
            dst_col=percore[c]['dst_col'],
            batch_col=np.ascontiguousarray(bc.reshape(NT, TILE).T),
            mask_col=np.ascontiguousarray(mk.reshape(NT, TILE).T),
            iota128=iota128, iota8=iota8, ident=ident,
        )
        for k, v in wts.items():
            m["w_" + k] = v
        ins.append(m)
    return ins


def run(cfg, x, edge_index, batch, params, trace=False, debug=False):
    from concourse.bass_utils import run_bass_kernel_spmd

    edge_index = np.asarray(edge_index)
    sched, percore = preprocess(cfg, edge_index)
    wts = derive_weights(cfg, params)
    nc = build(cfg, sched, wts, debug=debug)
    in_maps = _host_inputs(cfg, np.asarray(x, dtype=np.float32),
                           np.asarray(batch), percore, wts)
    res = run_bass_kernel_spmd(nc, in_maps, list(range(cfg.NC)), trace=trace)
    return res.results[0]["out"], res


def kernel(x, edge_index, batch, params):
    cfg = Cfg()
    out, _ = run(cfg, x, edge_index, batch, params)
    return np.asarray(out, dtype=np.float32)
